# revision 36
# baseline (speedup 1.0000x reference)
"""DeepSeek sparse attention on 8 Trainium2 NeuronCores.

Head-sharded (2 heads/core). v2 architecture — [q,s]-centric, engine-balanced:
  - indexer scores X computed per q-tile in PE-fp32, evacuated to SBUF (ACT)
  - top-32 threshold per query: 16 contiguous-block max8 + 4x(max8+match_replace)
    on DVE (fp32, selection-exact)
  - additive mask m' = (X < t32) * -30 via one fused tensor_scalar
    (DVE/GPSIMD split), accumulated into the attention-score PSUM by an
    identity-weight matmul (PE)
  - e = exp(S + m') on ACT with accum_out giving the softmax denominator free
  - w = e * recip(den) (DVE 4x-mode tensor_scalar), DMA-transposed to [s,q]
  - AV and out_proj in bf16; host sums the 8 per-core out_proj partials.
"""
import sys

sys.path.insert(0, '/opt/trn_rl_repo')
sys.path.insert(0, '/opt/pypackages')

import numpy as np
import ml_dtypes

BF16 = ml_dtypes.bfloat16

B, T, D = 1, 2048, 1024
H, DH, DI, KSEL = 16, 64, 32, 32
NCORES = 8
HPC = H // NCORES          # heads per core
NT = T // 128              # 16 query/key tiles
NK = D // 128              # 8 contraction chunks

_COMPILED = {}


def _install_drain_patch():
    import concourse.mybir as mybir
    from concourse.tile import TileContext
    from concourse.vector_clock import ScopedClock

    if getattr(TileContext, "_dsa_patched", False):
        return

    def _patched(self, tick_clock, wait_clock):
        nc = self.nc
        drain_inst = nc.sync.drain()
        wait_clock.add_sem_waits(
            drain_inst.ins, ScopedClock({None: tick_clock.global_clock})
        )
        si = drain_inst.ins.sync_info
        waits = list(si.on_wait or []) if si is not None else []
        if len(waits) > 1:
            drain_inst.ins.sync_info = mybir.SyncInfo(
                on_wait=waits[:1], on_update=list(si.on_update or [])
            )
            for i in range(1, len(waits)):
                extra = nc.sync.drain()
                extra.ins.sync_info = mybir.SyncInfo(
                    on_wait=waits[i:i + 1], on_update=[]
                )
        nc.all_engine_barrier()
        assert self.sems is not None
        popped = nc._tile_sem_poison_stack.pop()
        assert popped is self._sem_poison
        nc.clear_and_free_semaphores(list(self.sems.allocated().values()))
        nc.all_engine_barrier()

    TileContext._drain_and_barrier = _patched
    TileContext._dsa_patched = True


def _split_excess_waits(nc, limit=1):
    """walrus in this container rejects instructions with more sync waits
    than the ISA struct encodes; hoist excess waits onto standalone
    EventSemaphore instructions on the same engine, inserted just before."""
    import concourse.mybir as mybir

    n_new = 0
    for bb in nc.main_func.blocks:
        insts = bb.instructions
        i = 0
        while i < len(insts):
            ins = insts[i]
            si = ins.sync_info
            waits = list(si.on_wait or []) if si is not None else []
            if len(waits) > limit:
                ins.sync_info = mybir.SyncInfo(
                    on_wait=waits[:limit], on_update=list(si.on_update or []))
                pos = i
                for j in range(limit, len(waits), limit):
                    n_new += 1
                    w = mybir.InstEventSemaphore(
                        name=f"WSPLIT-{n_new}", ins=[], outs=[])
                    w.engine = ins.engine
                    w.sync_info = mybir.SyncInfo(
                        on_wait=waits[j:j + limit], on_update=[])
                    nc.register_instruction(w, overwrite=True)
                    insts.insert(pos, w)
                    pos += 1
                    i += 1
            i += 1
    return n_new


def _build_module():
    import concourse.bass as bass
    import concourse.mybir as mybir
    from concourse.tile import TileContext

    _install_drain_patch()
    dt = mybir.dt
    nc = bass.Bass()

    hsT_f32 = nc.declare_dram_parameter("hsT_f32", [D, T], dt.float32, isOutput=False)
    hsT_bf16 = nc.declare_dram_parameter("hsT_bf16", [D, T], dt.bfloat16, isOutput=False)
    A_cat = nc.declare_dram_parameter("A_cat", [D, 128], dt.float32, isOutput=False)
    Wqk_h0 = nc.declare_dram_parameter("Wqk_h0", [D, 128], dt.bfloat16, isOutput=False)
    Wqk_h1 = nc.declare_dram_parameter("Wqk_h1", [D, 128], dt.bfloat16, isOutput=False)
    Wv_cat = nc.declare_dram_parameter("Wv_cat", [D, 128], dt.bfloat16, isOutput=False)
    WoT_cat = nc.declare_dram_parameter("WoT_cat", [128, D], dt.bfloat16, isOutput=False)
    ident_in = nc.declare_dram_parameter("ident_in", [128, 128], dt.bfloat16, isOutput=False)
    out_part = nc.declare_dram_parameter("out_part", [T, D], dt.float32, isOutput=True)

    with TileContext(nc) as tc:
        # ----- core-lifetime SBUF state -----
        with tc.tile_pool(name="state", bufs=1) as st:
            # Iq2/Ik2: rows 0-32 h0, 32-64 h1, 64-96 h0 copy, 96-128 h1 copy —
            # the copies let two s-halves run as concurrent PE row-tiles.
            Iq2 = st.tile([128, T], dt.float32, tag="Iq2")
            Ik2 = st.tile([128, T], dt.float32, tag="Ik2")
            # per-head Q^T/K^T with rows 64-128 duplicating 0-64 (row-tile packing)
            QT0 = st.tile([128, T], dt.bfloat16, tag="QT0")
            QT1 = st.tile([128, T], dt.bfloat16, tag="QT1")
            KT0 = st.tile([128, T], dt.bfloat16, tag="KT0")
            KT1 = st.tile([128, T], dt.bfloat16, tag="KT1")
            # V per s-tile per head, cols 64-128 duplicating 0-64 (col-tile packing)
            VPD = st.tile([128, NT, 2, 128], dt.bfloat16, tag="VPD")
            WT = st.tile([128, NT, NT, 128], dt.bfloat16, tag="WT")   # w^T [p=s, j, i, q]
            ATcatT = st.tile([128, T], dt.bfloat16, tag="ATcatT")     # attn out^T, 2 heads
            wo = st.tile([128, D], dt.bfloat16, tag="wo")
            ident = st.tile([128, 128], dt.bfloat16, tag="ident")
            nc.sync.dma_start(out=wo[:], in_=WoT_cat[:])
            nc.sync.dma_start(out=ident[:], in_=ident_in[:])

            # ================= P0: projections =================
            with tc.tile_pool(name="p0s", bufs=2) as p0s, \
                 tc.tile_pool(name="p0h", bufs=1) as p0h, \
                 tc.tile_pool(name="p0w", bufs=1) as p0w, \
                 tc.tile_pool(name="p0p", bufs=1, space="PSUM") as p0p:
                hsb = p0h.tile([128, NK, T], dt.bfloat16, tag="hsb")
                nc.sync.dma_start(out=hsb[:], in_=hsT_bf16[:].rearrange("(c p) t -> p c t", p=128))
                a_w = p0w.tile([128, NK, 128], dt.float32, tag="a_w")
                qk0_w = p0w.tile([128, NK, 128], dt.bfloat16, tag="qk0_w")
                qk1_w = p0w.tile([128, NK, 128], dt.bfloat16, tag="qk1_w")
                v_w = p0w.tile([128, NK, 128], dt.bfloat16, tag="v_w")
                nc.sync.dma_start(out=a_w[:], in_=A_cat[:].rearrange("(c p) m -> p c m", p=128))
                nc.sync.dma_start(out=qk0_w[:], in_=Wqk_h0[:].rearrange("(c p) m -> p c m", p=128))
                nc.sync.dma_start(out=qk1_w[:], in_=Wqk_h1[:].rearrange("(c p) m -> p c m", p=128))
                nc.sync.dma_start(out=v_w[:], in_=Wv_cat[:].rearrange("(c p) m -> p c m", p=128))

                ip = p0p.tile([128, T], dt.float32, tag="ip")
                qp = p0p.tile([128, T], dt.float32, tag="qp")
                # pass A: I_cat (fp32) + QK_h0 (bf16)
                for k in range(NK):
                    hf = p0s.tile([128, T], dt.float32, tag="hf")
                    nc.sync.dma_start(out=hf[:], in_=hsT_f32[128 * k:128 * k + 128, :])
                    for n in range(4):
                        nc.tensor.matmul(ip[:, 512 * n:512 * n + 512],
                                         a_w[:, k, :], hf[:, 512 * n:512 * n + 512],
                                         start=(k == 0), stop=(k == NK - 1))
                    for n in range(4):
                        nc.tensor.matmul(qp[:, 512 * n:512 * n + 512],
                                         qk0_w[:, k, :], hsb[:, k, 512 * n:512 * n + 512],
                                         start=(k == 0), stop=(k == NK - 1))
                nc.scalar.copy(out=Iq2[0:64, :], in_=ip[0:64, :])
                nc.scalar.copy(out=Ik2[0:64, :], in_=ip[64:128, :])
                nc.scalar.copy(out=QT0[0:64, :], in_=qp[0:64, :])
                nc.scalar.copy(out=KT0[0:64, :], in_=qp[64:128, :])
                nc.sync.dma_start(out=Iq2[64:128, :], in_=Iq2[0:64, :])
                nc.sync.dma_start(out=Ik2[64:128, :], in_=Ik2[0:64, :])
                nc.sync.dma_start(out=QT0[64:128, :], in_=QT0[0:64, :])
                nc.sync.dma_start(out=KT0[64:128, :], in_=KT0[0:64, :])

                # pass B: QK_h1 + V (bf16)
                qp1 = p0p.tile([128, T], dt.float32, tag="ip")  # reuse slot
                vp_ps = p0p.tile([128, T], dt.float32, tag="qp")
                for k in range(NK):
                    for n in range(4):
                        nc.tensor.matmul(qp1[:, 512 * n:512 * n + 512],
                                         qk1_w[:, k, :], hsb[:, k, 512 * n:512 * n + 512],
                                         start=(k == 0), stop=(k == NK - 1))
                    for n in range(4):
                        nc.tensor.matmul(vp_ps[:, 512 * n:512 * n + 512],
                                         v_w[:, k, :], hsb[:, k, 512 * n:512 * n + 512],
                                         start=(k == 0), stop=(k == NK - 1))
                nc.scalar.copy(out=QT1[0:64, :], in_=qp1[0:64, :])
                nc.scalar.copy(out=KT1[0:64, :], in_=qp1[64:128, :])
                nc.sync.dma_start(out=QT1[64:128, :], in_=QT1[0:64, :])
                nc.sync.dma_start(out=KT1[64:128, :], in_=KT1[0:64, :])
                # V: rows 0:64 V_h0^T [dh, s], 64:128 V_h1^T -> DMA-transpose
                vt_b = p0w.tile([128, T], dt.bfloat16, tag="vt_b")
                nc.scalar.copy(out=vt_b[:], in_=vp_ps[:])
                vq = p0w.tile([128, NT, 128], dt.bfloat16, tag="vq")
                nc.sync.dma_start_transpose(out=vq[:], in_=vt_b[:])
                for h in range(2):
                    nc.sync.dma_start(out=VPD[:, :, h, 0:64],
                                      in_=vq[:, :, 64 * h:64 * h + 64])
                    nc.sync.dma_start(out=VPD[:, :, h, 64:128],
                                      in_=vq[:, :, 64 * h:64 * h + 64])

            # ================= per-head main loops =================
            with tc.tile_pool(name="sx", bufs=4) as sx, \
                 tc.tile_pool(name="se", bufs=3) as se, \
                 tc.tile_pool(name="sw", bufs=3) as sw, \
                 tc.tile_pool(name="sc", bufs=4) as scp, \
                 tc.tile_pool(name="px", bufs=1, space="PSUM") as px, \
                 tc.tile_pool(name="ps", bufs=2, space="PSUM") as pss:
                for h in range(HPC):
                    QTh = QT0 if h == 0 else QT1
                    KTh = KT0 if h == 0 else KT1
                    for i in range(NT):
                        # ---- indexer scores X [q, s] fp32, two packed row-tiles ----
                        xp = px.tile([128, T], dt.float32, tag="xp")
                        for n in range(2):
                            nc.tensor.matmul(
                                xp[:, 512 * n:512 * n + 512],
                                Iq2[32 * h:32 * h + 32, 128 * i:128 * i + 128],
                                Ik2[32 * h:32 * h + 32, 512 * n:512 * n + 512],
                                tile_position=(32 * h, 0))
                            nc.tensor.matmul(
                                xp[:, 1024 + 512 * n:1024 + 512 * n + 512],
                                Iq2[64 + 32 * h:64 + 32 * h + 32, 128 * i:128 * i + 128],
                                Ik2[64 + 32 * h:64 + 32 * h + 32,
                                    1024 + 512 * n:1024 + 512 * n + 512],
                                tile_position=(64 + 32 * h, 0))
                        xs = sx.tile([128, T], dt.float32, tag="xs")
                        nc.scalar.copy(out=xs[:], in_=xp[:])

                        # ---- top-32 threshold on DVE ----
                        cand = scp.tile([128, 128], dt.float32, tag="cand")
                        for j in range(16):
                            nc.vector.max(out=cand[:, 8 * j:8 * j + 8],
                                          in_=xs[:, 128 * j:128 * j + 128])
                        mx = scp.tile([128, 8], dt.float32, tag="mx")
                        for r in range(4):
                            nc.vector.max(out=mx[:], in_=cand[:])
                            if r < 3:
                                nc.vector.match_replace(out=cand[:], in_to_replace=mx[:],
                                                        in_values=cand[:], imm_value=-1e30)

                        # ---- S [q, s] bf16, two packed row-tiles per half ----
                        e = se.tile([128, T], dt.bfloat16, tag="e")
                        spA = pss.tile([128, 1024], dt.float32, tag="sp")
                        spB = pss.tile([128, 1024], dt.float32, tag="sp")
                        for n in range(2):
                            nc.tensor.matmul(spA[:, 512 * n:512 * n + 512],
                                             QTh[0:64, 128 * i:128 * i + 128],
                                             KTh[0:64, 512 * n:512 * n + 512],
                                             tile_position=(0, 0))
                            nc.tensor.matmul(spB[:, 512 * n:512 * n + 512],
                                             QTh[64:128, 128 * i:128 * i + 128],
                                             KTh[64:128, 1024 + 512 * n:1024 + 512 * n + 512],
                                             tile_position=(64, 0))
                        nc.scalar.activation(out=e[:, 0:1024], in_=spA[:],
                                             func=mybir.ActivationFunctionType.Exp)
                        nc.scalar.activation(out=e[:, 1024:2048], in_=spB[:],
                                             func=mybir.ActivationFunctionType.Exp)

                        # ---- fused mask+apply: w = (X >= t32) * e, den = sum ----
                        w = sw.tile([128, T], dt.bfloat16, tag="w")
                        dsum = scp.tile([128, 1], dt.float32, tag="dsum")
                        nc.vector.scalar_tensor_tensor(
                            out=w[:], in0=xs[:], scalar=mx[:, 7:8], in1=e[:],
                            op0=mybir.AluOpType.is_ge, op1=mybir.AluOpType.mult,
                            accum_out=dsum[:])
                        rd = scp.tile([128, 1], dt.float32, tag="rd")
                        nc.vector.reciprocal(rd[:], dsum[:])
                        wn = sw.tile([128, T], dt.bfloat16, tag="wn")
                        if i % 2 == 0:
                            nc.scalar.activation(out=wn[:], in_=w[:],
                                                 func=mybir.ActivationFunctionType.Copy,
                                                 bias=0.0, scale=rd[:])
                        else:
                            nc.vector.tensor_scalar(wn[:], w[:], rd[:], scalar2=None,
                                                    op0=mybir.AluOpType.mult)
                        nc.sync.dma_start_transpose(out=WT[:, :, i, :], in_=wn[:])

                        # ---- AV pass p needs only WT i-columns 8p..8p+8: issue
                        # p=0 mid-loop (after i=7) so it overlaps the back half ----
                        if i in (7, NT - 1):
                            p = 0 if i == 7 else 1
                            av = pss.tile([128, 1024], dt.float32, tag="sp")
                            for j in range(NT):
                                nc.tensor.matmul(
                                    av[0:64, 0:512],
                                    VPD[:, j, h, 0:64],
                                    WT[:, j, 8 * p:8 * p + 4, :],
                                    tile_position=(0, 0),
                                    start=(j == 0), stop=(j == NT - 1))
                                nc.tensor.matmul(
                                    av[64:128, 0:512],
                                    VPD[:, j, h, 64:128],
                                    WT[:, j, 8 * p + 4:8 * p + 8, :],
                                    tile_position=(0, 64),
                                    start=(j == 0), stop=(j == NT - 1))
                            nc.scalar.copy(out=ATcatT[64 * h:64 * h + 64,
                                                      1024 * p:1024 * p + 512],
                                           in_=av[0:64, 0:512])
                            nc.scalar.copy(out=ATcatT[64 * h:64 * h + 64,
                                                      1024 * p + 512:1024 * p + 1024],
                                           in_=av[64:128, 0:512])

                # ---- out_proj (shares sp PSUM slots; overlaps final AV) ----
                for i in range(NT):
                    op = pss.tile([128, D], dt.float32, tag="sp")
                    for n in range(2):
                        nc.tensor.matmul(op[:, 512 * n:512 * n + 512],
                                         ATcatT[:, 128 * i:128 * i + 128],
                                         wo[:, 512 * n:512 * n + 512])
                    ob = sw.tile([128, D], dt.float32, tag="ob")
                    nc.scalar.copy(out=ob[:], in_=op[:])
                    nc.sync.dma_start(out=out_part[128 * i:128 * i + 128, :], in_=ob[:])

    _split_excess_waits(nc, limit=1)
    return nc


def _prep_inputs(hidden_states, Wq, Wk, Wv, Wo, idx_wq, idx_wk):
    hs = np.asarray(hidden_states[0], np.float32)          # [T, D]
    hsT = np.ascontiguousarray(hs.T)                       # [D, T]
    hsT_b = hsT.astype(BF16)
    ident = np.eye(128, dtype=np.float32).astype(BF16)
    maps = []
    for c in range(NCORES):
        h0, h1 = 2 * c, 2 * c + 1
        Aq_parts, Ak_parts = [], []
        for hh in (h0, h1):
            Wq_h = Wq[64 * hh:64 * hh + 64, :].astype(np.float64)    # [64, D]
            Wk_h = Wk[64 * hh:64 * hh + 64, :].astype(np.float64)
            Aq_parts.append((Wq_h.T @ idx_wq[hh].astype(np.float64)).astype(np.float32))
            Ak_parts.append((Wk_h.T @ idx_wk[hh].astype(np.float64)).astype(np.float32))
        A_cat = np.concatenate(Aq_parts + Ak_parts, axis=1)  # [D, 128]

        def qk_chain(hh):
            Wq_h = Wq[64 * hh:64 * hh + 64, :]
            Wk_h = Wk[64 * hh:64 * hh + 64, :]
            return np.concatenate(
                [(Wq_h.T / np.sqrt(DH)).astype(BF16), Wk_h.T.astype(BF16)], axis=1)

        Wv_cat = np.concatenate(
            [Wv[64 * h0:64 * h0 + 64, :].T, Wv[64 * h1:64 * h1 + 64, :].T],
            axis=1).astype(BF16)                           # [D, 128]
        WoT_cat = np.ascontiguousarray(Wo[:, 64 * h0:64 * h0 + 128].T).astype(BF16)

        maps.append({
            "hsT_f32": hsT,
            "hsT_bf16": hsT_b,
            "A_cat": A_cat,
            "Wqk_h0": qk_chain(h0),
            "Wqk_h1": qk_chain(h1),
            "Wv_cat": Wv_cat,
            "WoT_cat": WoT_cat,
            "ident_in": ident,
        })
    return maps


def kernel(hidden_states, Wq, Wk, Wv, Wo, idx_wq, idx_wk):
    from concourse.bass_utils import run_bass_kernel_spmd

    if "nc" not in _COMPILED:
        _COMPILED["nc"] = _build_module()
    nc = _COMPILED["nc"]

    in_maps = _prep_inputs(np.asarray(hidden_states), np.asarray(Wq),
                           np.asarray(Wk), np.asarray(Wv), np.asarray(Wo),
                           np.asarray(idx_wq), np.asarray(idx_wk))
    res = run_bass_kernel_spmd(nc, in_maps, core_ids=list(range(NCORES)))
    out = np.zeros((T, D), np.float32)
    for c in range(NCORES):
        out += np.asarray(res.results[c]["out_part"], np.float32)
    return out.reshape(B, T, D)
